# revision 31
# baseline (speedup 1.0000x reference)
# Multi-head attention with RoPE, tensor-parallel over heads on 8 NeuronCores.
# v2: 512-wide tq blocks with a composite scores PSUM tile (both heads side
# by side -> ONE exp instruction per key chunk), 6-bank attention drum, and
# the b=1 q/k projections woven through the b=0 attention drum so the PE's
# spare cycles during the ACT-bound drum do useful work.
#
# Layouts (all matmul inputs bf16, fp32 accumulation):
#   xT   [D, T]    : x transposed on host; contraction d on partitions.
#   q2,k2 [E=128,T]: rope'd activations in place (h0 rows 0-63, h1 64-127).
#   v_sb [tk 128, chunk, head, 65] = [v_h | 1] per head (ones -> denom row).
#   scores^T       : composite PSUM tile [128, 1024] per tkc: cols 0-511 =
#                    h0 (tile (0,0), contracts partitions 0-63), cols
#                    512-1023 = h1 (tile (64,0)) -> the two matmuls run
#                    CONCURRENTLY on disjoint PE row groups; ONE 1024-wide
#                    exp instruction serves both heads.
#   attn@v         : ot_h [65, 512] PSUM accumulators (1 bank each).
#   norm           : denom row DVE-copied out, reciprocal + gpsimd broadcast,
#                    one [128,512] DVE mul into on_sb.
#   out-proj       : y[t, :] = on^T @ woT per 128-token chunk, psum halves
#                    from the shared weave pool, bf16 staged, DMA'd out.

import numpy as np
import ml_dtypes

import concourse.bass as bass
import concourse.mybir as mybir
import concourse.tile as tile
from concourse import bacc

B, N, D, H = 2, 2048, 1024, 16
HD = 64
T = B * N                 # 4096 tokens
NCORES = 8
HPC = H // NCORES         # 2 heads per core
E = HPC * HD              # 128 per-core projection columns
KD = D // 128             # 8 contraction tiles for d
ROPE_BASE = 10000.0

BF = mybir.dt.bfloat16
F32 = mybir.dt.float32

TQB = 512                 # tq block (psum width per head)
NB = N // TQB             # 4 blocks per batch
NKC = N // 128            # 16 key chunks per batch
EW = 2 * TQB              # composite exp width (both heads)


def build_nc():
    nc = bacc.Bacc(trn_type="TRN2", target_bir_lowering=False, debug=False)

    xT = nc.dram_tensor("xT", [D, T], BF, kind="ExternalInput").ap()
    wqT = nc.dram_tensor("wqT", [D, E], BF, kind="ExternalInput").ap()
    wkT = nc.dram_tensor("wkT", [D, E], BF, kind="ExternalInput").ap()
    wvT = nc.dram_tensor("wvT", [D, E], BF, kind="ExternalInput").ap()
    woT = nc.dram_tensor("woT", [E, D], BF, kind="ExternalInput").ap()
    bcol = nc.dram_tensor("bcol", [E, 3], F32, kind="ExternalInput").ap()
    cosb = nc.dram_tensor("cosb", [E, N], BF, kind="ExternalInput").ap()
    sinb = nc.dram_tensor("sinb", [E, N], BF, kind="ExternalInput").ap()
    rotT = nc.dram_tensor("rotT", [E, E], BF, kind="ExternalInput").ap()
    idT = nc.dram_tensor("idT", [E, E], BF, kind="ExternalInput").ap()
    y = nc.dram_tensor("y", [T, D], BF, kind="ExternalOutput").ap()

    with tile.TileContext(nc) as tc:
        _build(tc, nc, xT, wqT, wkT, wvT, woT, bcol, cosb, sinb,
               rotT, idT, y)
    nc.compile()
    return nc


def _build(tc, nc, xT, wqT, wkT, wvT, woT, bcol, cosb, sinb,
           rotT, idT, y):
    with (
        tc.tile_pool(name="consts", bufs=1) as consts,
        tc.tile_pool(name="xbig", bufs=1) as xbig,
        tc.tile_pool(name="ebig", bufs=1) as ebig,
        tc.tile_pool(name="acts", bufs=1) as acts,
        tc.tile_pool(name="small", bufs=3) as small,
    ):
        # ---- constants / weights ----
        # DMA order tracks compute order: phase 1a projects K of slice 0
        # first, so wk and the first half-slice of x go out first; the
        # slice-0 K projection runs in 256-token halves so the PE starts
        # as soon as the first half lands.
        wq_sb = consts.tile([128, KD, E], BF, tag="wq")
        wk_sb = consts.tile([128, KD, E], BF, tag="wk")
        wv_sb = consts.tile([128, KD, E], BF, tag="wv")
        nc.sync.dma_start(out=wk_sb, in_=wkT.rearrange("(k p) e -> p k e", p=128))
        x_sb = xbig.tile([128, KD, T], BF, tag="big")
        xTr = xT.rearrange("(k p) t -> p k t", p=128)
        nc.sync.dma_start(out=x_sb[:, :, 0:256], in_=xTr[:, :, 0:256])
        nc.sync.dma_start(out=x_sb[:, :, 256:512], in_=xTr[:, :, 256:512])
        nc.sync.dma_start(out=wv_sb, in_=wvT.rearrange("(k p) e -> p k e", p=128))
        nc.sync.dma_start(out=wq_sb, in_=wqT.rearrange("(k p) e -> p k e", p=128))
        nc.sync.dma_start(out=x_sb[:, :, 512:1024], in_=xTr[:, :, 512:1024])
        wo_sb = consts.tile([E, D], BF, tag="wo")
        nc.sync.dma_start(out=wo_sb, in_=woT)
        bcol_sb = consts.tile([E, 3], F32, tag="bcol")
        nc.sync.dma_start(out=bcol_sb, in_=bcol)
        cos_sb = consts.tile([E, N], BF, tag="cos")
        sin_sb = consts.tile([E, N], BF, tag="sin")
        nc.sync.dma_start(out=cos_sb, in_=cosb)
        nc.sync.dma_start(out=sin_sb, in_=sinb)
        rot_sb = consts.tile([E, E], BF, tag="rot")
        nc.sync.dma_start(out=rot_sb, in_=rotT)
        id_sb = consts.tile([E, E], BF, tag="idT")
        nc.sync.dma_start(out=id_sb, in_=idT)

        for ci in range(2, T // 512):
            nc.sync.dma_start(out=x_sb[:, :, ci * 512:(ci + 1) * 512],
                              in_=xTr[:, :, ci * 512:(ci + 1) * 512])

        # ---- persistent activations ----
        q2 = acts.tile([E, T], BF, tag="q2")
        k2 = acts.tile([E, T], BF, tag="k2")
        v_sb = acts.tile([128, T // 128, HPC, HD + 1], BF, tag="v_sb")
        on_sb = acts.tile([E, B, N], BF, tag="on_sb")

        nc.vector.memset(v_sb[:, :, :, HD:HD + 1], 1.0)

        # ---------- projection emitters (used in phase 1a and the weave) ----
        def proj_qk_slice(ps_pool, psr_pool, ci, dst, w, bc, granules=None):
            # q or k projection for 512-token slice ci, rope'd in place.
            # When `granules` is a list, work is appended as deferred items.
            # PSUM tiles are allocated inside the granule bodies so pool
            # slot rotation matches emission order.
            sl = slice(ci * 512, (ci + 1) * 512)
            npos = (ci * 512) % N
            tsl = slice(npos, npos + 512)
            cell = {}

            def part_mm(k):
                def emit():
                    if k == 0:
                        cell["ps"] = ps_pool.tile([128, 512], F32,
                                                  tag="ps_qk",
                                                  name=f"ps_{bc}_{ci}")
                    ps = cell["ps"]
                    nc.tensor.matmul(ps, w[:, k, :], x_sb[:, k, sl],
                                     start=(k == 0), stop=(k == KD - 1))
                    if k == KD - 1:
                        nc.vector.tensor_scalar_add(
                            dst[:, sl], ps, bcol_sb[:, bc:bc + 1])
                return emit

            def part_rope():
                psr = psr_pool.tile([128, 512], F32, tag="ps_qk",
                                    name=f"psr_{bc}_{ci}")
                nc.tensor.matmul(psr, rot_sb, dst[:, sl], start=True, stop=True)
                t1 = small.tile([128, 512], BF, tag="rope_t1")
                nc.vector.tensor_mul(t1, dst[:, sl], cos_sb[:, tsl])
                t2 = small.tile([128, 512], BF, tag="rope_t2")
                nc.vector.tensor_mul(t2, psr, sin_sb[:, tsl])
                nc.vector.tensor_add(dst[:, sl], t1, t2)

            if granules is None:
                for k in range(KD):
                    part_mm(k)()
                part_rope()
            else:
                granules.extend([part_mm(k) for k in range(KD)])
                granules.append(part_rope)

        def proj_v_slice(ps_pool, pst_pool, ci):
            sl = slice(ci * 512, (ci + 1) * 512)
            psv = ps_pool.tile([128, 512], F32, tag="ps_qk", name=f"psv_{ci}")
            for k in range(KD):
                nc.tensor.matmul(psv, wv_sb[:, k, :], x_sb[:, k, sl],
                                 start=(k == 0), stop=(k == KD - 1))
            vts = small.tile([128, 512], BF, tag="vts")
            nc.vector.tensor_scalar_add(vts, psv, bcol_sb[:, 2:3])
            for s in range(4):
                cv = ci * 4 + s
                pst = pst_pool.tile([128, 128], BF, tag="ps_t",
                                    name=f"pst_{ci}_{s}")
                nc.tensor.transpose(pst, vts[:, s * 128:(s + 1) * 128], id_sb)
                nc.vector.tensor_copy(v_sb[:, cv, :, 0:HD], pst)

        # ================= phase 1a =================
        # b=0 projections (slices 0-3) fully, plus v for b=1 (slices 4-7,
        # transposes need their own psum which the drum can't spare).
        with (
            tc.tile_pool(name="ps_qk", bufs=2, space="PSUM") as ps_qk,
            tc.tile_pool(name="ps_r", bufs=2, space="PSUM") as ps_r,
            tc.tile_pool(name="ps_t", bufs=2, space="PSUM") as ps_t,
        ):
            # slice-0 K projection in 256-token halves (prime the PE while
            # the second half of x slice 0 is still in flight)
            psk0 = ps_qk.tile([128, 512], F32, tag="ps_qk", name="psk0")
            for half in range(2):
                hsl = slice(half * 256, (half + 1) * 256)
                for k in range(KD):
                    nc.tensor.matmul(psk0[:, hsl], wk_sb[:, k, :],
                                     x_sb[:, k, hsl],
                                     start=(k == 0), stop=(k == KD - 1))
            nc.vector.tensor_scalar_add(k2[:, 0:512], psk0, bcol_sb[:, 1:2])
            psr0 = ps_r.tile([128, 512], F32, tag="ps_qk", name="psr0")
            nc.tensor.matmul(psr0, rot_sb, k2[:, 0:512], start=True, stop=True)
            t10 = small.tile([128, 512], BF, tag="rope_t1", name="t10")
            nc.vector.tensor_mul(t10, k2[:, 0:512], cos_sb[:, 0:512])
            t20 = small.tile([128, 512], BF, tag="rope_t2", name="t20")
            nc.vector.tensor_mul(t20, psr0, sin_sb[:, 0:512])
            nc.vector.tensor_add(k2[:, 0:512], t10, t20)

            for ci in range(4):
                if ci > 0:
                    proj_qk_slice(ps_qk, ps_r, ci, k2, wk_sb, 1)
                proj_v_slice(ps_qk, ps_t, ci)
                proj_qk_slice(ps_qk, ps_r, ci, q2, wq_sb, 0)
            for ci in range(4, 8):
                proj_v_slice(ps_qk, ps_t, ci)

        # ========= phase 2: attention drum + woven work =========
        with (
            tc.tile_pool(name="ps_sc", bufs=2, space="PSUM") as ps_sc,
            tc.tile_pool(name="ps_o", bufs=2, space="PSUM") as ps_o,
            tc.tile_pool(name="ps_w", bufs=2, space="PSUM") as ps_w,
        ):
            critical = []   # b=1 q/k projection granules (must finish in b0)
            pending = []    # norm / out-proj items (may spill)
            _state = {"drain": False}

            # enqueue the b=1 q/k projection granules
            for ci in range(4, 8):
                proj_qk_slice(ps_w, ps_w, ci, k2, wk_sb, 1, granules=critical)
            for ci in range(4, 8):
                proj_qk_slice(ps_w, ps_w, ci, q2, wq_sb, 0, granules=critical)

            def norm_item(b, qb, ou2f, rss):
                def emit():
                    for h in range(HPC):
                        rc = small.tile([1, TQB], F32, tag="recip", bufs=2,
                                        name=f"rc_{b}_{qb}_{h}")
                        nc.vector.reciprocal_approx_fast(out=rc, in_=rss[h])
                        rbs = small.tile([128, TQB], F32, tag="recipb",
                                         bufs=2, name=f"rbs_{b}_{qb}_{h}")
                        nc.gpsimd.partition_broadcast(
                            out_ap=rbs, in_ap=rc, channels=128)
                        hsl = slice(h * HD, (h + 1) * HD)
                        nc.vector.tensor_mul(
                            on_sb[hsl, b, qb * TQB:(qb + 1) * TQB],
                            ou2f[hsl, :], rbs[hsl, :])
                    for ci in range(qb * 4, qb * 4 + 4):
                        ycell = {}
                        for eo in range(D // 512):
                            pending.append(y_item(b, ci, eo, ycell))
                return emit

            def y_item(b, ci, eo, cell):
                # one 512-col half of the out-projection for 128 tokens
                def emit():
                    if eo == 0:
                        cell["ysb"] = small.tile([128, D], BF, tag="ysb",
                                                 bufs=4,
                                                 name=f"ysb_{b}_{ci}")
                    ysb = cell["ysb"]
                    psy = ps_w.tile([128, 512], F32, tag="ps_qk",
                                    name=f"psy_{b}_{ci}_{eo}")
                    nc.tensor.matmul(
                        psy,
                        on_sb[:, b, ci * 128:(ci + 1) * 128],
                        wo_sb[:, eo * 512:(eo + 1) * 512],
                        start=True, stop=True)
                    if _state["drain"] and (ci + eo) % 2 == 1:
                        nc.scalar.copy(ysb[:, eo * 512:(eo + 1) * 512], psy)
                    else:
                        nc.vector.tensor_copy(
                            ysb[:, eo * 512:(eo + 1) * 512], psy)
                    if eo == D // 512 - 1:
                        nc.sync.dma_start(
                            out=y[b * N + ci * 128: b * N + (ci + 1) * 128, :],
                            in_=ysb)
                return emit

            def pop_item():
                if critical:
                    critical.pop(0)()
                elif pending:
                    pending.pop(0)()

            # ---- flat global software pipeline over (b, qb, tkc) ----
            # Per iteration i: emit scores(i+1) [one period AHEAD of the
            # exp that consumes it, so ACT never waits on the PE FIFO],
            # exp(i), attn(i-1), block-end evacuation, then one deferred
            # item. This keeps the exp drum gap-free across block
            # boundaries too.
            triples = [(b, qb, tkc)
                       for b in range(B) for qb in range(NB)
                       for tkc in range(NKC)]
            NTR = len(triples)
            blk_state = {}
            sc_of = {}

            def emit_scores(i):
                b, qb, tkc = triples[i]
                tq0 = b * N + qb * TQB
                sc = ps_sc.tile([128, EW], F32, tag="ps_sc",
                                name=f"sc_{b}_{qb}_{tkc}")
                sc_of[i] = sc
                # h0 -> cols 0-511 (PE tile (0,0)), h1 -> cols 512-1023
                # (PE tile (64,0)): concurrent matmuls.
                for h in range(HPC):
                    nc.tensor.matmul(
                        sc[:, h * TQB:(h + 1) * TQB],
                        k2[h * HD:(h + 1) * HD,
                           b * N + tkc * 128: b * N + (tkc + 1) * 128],
                        q2[h * HD:(h + 1) * HD, tq0: tq0 + TQB],
                        start=True, stop=True)

            NEARLY = 4   # first chunks use a decoupled double-buffered tile

            def exp_slot(st, b, qb, tkc):
                if tkc < NEARLY:
                    if "exp0" not in st:
                        st["exp0"] = small.tile([128, NEARLY, EW], BF,
                                                tag="exp0", bufs=2,
                                                name=f"exp0_{b}_{qb}")
                    return st["exp0"][:, tkc, :]
                if "exp_t" not in st:
                    st["exp_t"] = ebig.tile([128, NKC, EW], BF, tag="exp",
                                            name=f"exp_{b}_{qb}")
                return st["exp_t"][:, tkc, :]

            def emit_exp(i):
                b, qb, tkc = triples[i]
                st = blk_state.setdefault((b, qb), {})
                nc.scalar.activation(
                    out=exp_slot(st, b, qb, tkc), in_=sc_of.pop(i),
                    func=mybir.ActivationFunctionType.Exp,
                    scale=float(HD) ** -0.5)

            def emit_attn(i):
                b, qb, tkc = triples[i]
                st = blk_state[(b, qb)]
                if tkc == 0:
                    st["ots"] = [ps_o.tile([HD + 1, TQB], F32, tag="ps_o",
                                           name=f"ot_{b}_{qb}_{h}")
                                 for h in range(HPC)]
                src = (st["exp0"][:, tkc, :] if tkc < NEARLY
                       else st["exp_t"][:, tkc, :])
                for h in range(HPC):
                    nc.tensor.matmul(
                        st["ots"][h], v_sb[:, b * NKC + tkc, h, :],
                        src[:, h * TQB:(h + 1) * TQB],
                        start=(tkc == 0), stop=(tkc == NKC - 1))
                if tkc == NKC - 1:
                    emit_evac(b, qb, st)

            def emit_evac(b, qb, st):
                # fast PSUM evacuation (DVE); recip chain deferred
                ots = st["ots"]
                ou2f = small.tile([128, TQB], F32, tag="ou", bufs=2,
                                  name=f"ou_{b}_{qb}")
                nc.vector.tensor_copy(ou2f[0:HD, :], ots[0][0:HD, :])
                nc.vector.tensor_copy(ou2f[HD:2 * HD, :], ots[1][0:HD, :])
                rss = []
                for h in range(HPC):
                    rs = small.tile([1, TQB], F32, tag="rs", bufs=4,
                                    name=f"rs_{b}_{qb}_{h}")
                    nc.vector.tensor_copy(rs, ots[h][HD:HD + 1, :])
                    rss.append(rs)
                pending.append(norm_item(b, qb, ou2f, rss))

            for i in range(NTR):
                if i == 0:
                    emit_scores(0)
                if i + 1 < NTR:
                    emit_scores(i + 1)
                emit_exp(i)
                if i > 0:
                    emit_attn(i - 1)
                if i >= 1:
                    pop_item()
                    # drain the backlog harder when it exceeds remaining slots
                    if len(critical) + len(pending) > NTR - i:
                        pop_item()
            emit_attn(NTR - 1)

            _state["drain"] = True
            while critical:
                critical.pop(0)()
            while pending:
                pending.pop(0)()


def _host_inputs(x, Wq, Wk, Wv, Wo, bq, bk, bv, bo):
    """Build the 8 per-core input maps (host-side sharding + layout prep)."""
    bf16 = ml_dtypes.bfloat16
    xTh = np.ascontiguousarray(x.reshape(T, D).T).astype(bf16)

    i = (np.arange(E) % HD) % (HD // 2)
    inv_freq = ROPE_BASE ** (-2.0 * i / HD)  # [E]
    ang = np.arange(N)[None, :] * inv_freq[:, None]          # [E, N]
    cosb = np.cos(ang).astype(bf16)
    sinb = np.sin(ang).astype(bf16)

    P = np.zeros((E, E), dtype=np.float32)
    for h in range(HPC):
        for j in range(HD // 2):
            P[h * HD + j, h * HD + j + HD // 2] = -1.0
            P[h * HD + j + HD // 2, h * HD + j] = 1.0
    rotT = np.ascontiguousarray(P.T).astype(bf16)
    idT = np.eye(E, dtype=np.float32).astype(bf16)

    in_maps = []
    for c in range(NCORES):
        sl = slice(c * E, (c + 1) * E)
        in_maps.append({
            "xT": xTh,
            "wqT": np.ascontiguousarray(Wq[sl, :].T).astype(bf16),
            "wkT": np.ascontiguousarray(Wk[sl, :].T).astype(bf16),
            "wvT": np.ascontiguousarray(Wv[sl, :].T).astype(bf16),
            "woT": np.ascontiguousarray(Wo[:, sl].T).astype(bf16),
            "bcol": np.stack([bq[sl], bk[sl], bv[sl]], axis=1)
                .astype(np.float32),
            "cosb": cosb,
            "sinb": sinb,
            "rotT": rotT,
            "idT": idT,
        })
    return in_maps


_NC = None


def kernel(x, Wq, Wk, Wv, Wo, bq, bk, bv, bo):
    from concourse.bass_utils import run_bass_kernel_spmd

    global _NC
    if _NC is None:
        _NC = build_nc()
    bo = np.asarray(bo, dtype=np.float32)
    in_maps = _host_inputs(np.asarray(x, dtype=np.float32),
                           np.asarray(Wq, dtype=np.float32),
                           np.asarray(Wk, dtype=np.float32),
                           np.asarray(Wv, dtype=np.float32),
                           np.asarray(Wo, dtype=np.float32),
                           np.asarray(bq, dtype=np.float32),
                           np.asarray(bk, dtype=np.float32),
                           np.asarray(bv, dtype=np.float32),
                           bo)
    res = run_bass_kernel_spmd(_NC, in_maps, core_ids=list(range(NCORES)))
    out = np.zeros((T, D), dtype=np.float32)
    for r in res.results:
        out += np.asarray(r["y"], dtype=np.float32)
    out += bo[None, :]
    return out.reshape(B, N, D)
